# revision 1
# baseline (speedup 1.0000x reference)
"""Trainium2 Bass kernel v2 for nn_AttentionLayer — PE front-end, dense masked output.

Same math as v1 (see kernel.py docstring): per core the output slice is
    out[b, v'] = occ[b, v'] * leaky_relu(t[v'] + s[b])
with t = table_shard @ a_w, s = attr_emb @ a_a.

v2 engine plan (per 640-wide vocab strip):
  PE   : transpose table tiles to [d, v] and matmul with a column-replicated
         a_w weight matrix -> PSUM holds t replicated across all 128
         partitions (no DRAM round-trip, no broadcast DMA, no DVE matvec)
  ACT  : ep = t + s[b] (reads PSUM, per-partition bias); tq = 0.2 * q
  DVE  : PSUM->SBUF copies of the transposed table; q = ep * occ
         (mixed-dtype f32 x int8 multiply, no cast pass)
  POOL : o = max(q, tq)  (= occ * leaky_relu)
  SYNC : all DMA
"""

import numpy as np

import concourse.bass as bass
import concourse.tile as tile
from concourse import bacc, mybir
from concourse.bass_utils import run_bass_kernel_spmd

B = 256
L = 512
V = 50257
DW = 256
DA = 256
ALPHA = 0.2

NCORES = 8
VS = 6400          # vocab span per core
SW = 1280          # strip width (10 x 128)
NS = VS // SW      # 5 strips
GP = SW // 128     # 5 v-groups per strip

_CACHE = {}


def _build():
    if "nc" in _CACHE:
        return _CACHE["nc"]
    f32 = mybir.dt.float32
    i8 = mybir.dt.int8

    nc = bacc.Bacc("TRN2", target_bir_lowering=False, debug=False)
    f32r = mybir.dt.float32r
    tblT = nc.declare_dram_parameter("tblT", [DW, VS], f32r, isOutput=False)
    occ = nc.declare_dram_parameter("occ", [B, VS], i8, isOutput=False)
    awb = nc.declare_dram_parameter("awb", [DW, 128], f32r, isOutput=False)
    aa = nc.declare_dram_parameter("aa", [128, DA], f32, isOutput=False)
    attr = nc.declare_dram_parameter("attr", [B, DA], f32, isOutput=False)
    out = nc.declare_dram_parameter("out", [B, VS], f32, isOutput=True)

    with tile.TileContext(nc) as tc:
        with (
            tc.tile_pool(name="sb", bufs=1) as sb,
            tc.tile_pool(name="tp", bufs=3) as tp,
            tc.tile_pool(name="bk", bufs=4) as bk,
            tc.tile_pool(name="pst", bufs=2, space="PSUM") as pst,
        ):
            # a_w as column-replicated weights, one [128, 128] tile per d-half
            awb_t = sb.tile([128, 2 * 128], f32r, tag="awb")
            nc.sync.dma_start(
                awb_t[:].rearrange("p (dh m) -> p dh m", m=128),
                awb.ap().rearrange("(dh p) m -> p dh m", p=128),
            )
            aa_t = sb.tile([128, DA], f32, tag="aa")
            nc.sync.dma_start(aa_t[:], aa.ap())

            # ---- s = attr_emb @ a_a  (s_sb[:, h] holds b = h*128 + p) ----
            s_sb = sb.tile([128, 2], f32, tag="s")
            for h in range(2):
                at = sb.tile([128, DA], f32, tag=f"attr{h}")
                nc.sync.dma_start(at[:], attr.ap()[h * 128 : (h + 1) * 128, :])
                pa = sb.tile([128, DA], f32, tag=f"pa{h}")
                nc.vector.tensor_tensor(
                    out=pa[:], in0=at[:], in1=aa_t[:], op=mybir.AluOpType.mult
                )
                nc.vector.tensor_reduce(
                    out=s_sb[:, h : h + 1],
                    in_=pa[:],
                    axis=mybir.AxisListType.X,
                    op=mybir.AluOpType.add,
                )

            s08 = sb.tile([128, 2], f32, tag="s08")
            nc.vector.tensor_scalar_mul(s08[:], s_sb[:], 1.0 - ALPHA)
            s02 = sb.tile([128, 2], f32, tag="s02")
            nc.vector.tensor_scalar_mul(s02[:], s_sb[:], ALPHA)

            # ---- hoisted strip loads (scheduler pulls these as slots free) ----
            tTts, m8s = [], []
            for si in range(NS):
                cs = slice(si * SW, (si + 1) * SW)
                tTt = tp.tile([128, 2 * SW], f32r, tag="tblT")
                nc.sync.dma_start(
                    tTt[:].rearrange("p (dh v) -> p dh v", v=SW),
                    tblT.ap().rearrange("(dh p) v -> p dh v", p=128)[:, :, cs],
                )
                tTts.append(tTt)
                m8 = bk.tile([128, 2 * SW], i8, tag="m8")
                nc.sync.dma_start(
                    m8[:].rearrange("p (h v) -> p h v", v=SW),
                    occ.ap().rearrange("(h p) v -> p h v", p=128)[:, :, cs],
                )
                m8s.append(m8)

            # ---- per strip ----
            for si in range(NS):
                cs = slice(si * SW, (si + 1) * SW)
                tTt = tTts[si]
                m8 = m8s[si]
                # t (replicated across partitions) = awb^T @ tblT, K-accumulated
                pt = pst.tile([128, SW], f32, tag="pt")
                for dh in range(2):
                    for n0, n1 in ((0, 512), (512, 1024), (1024, SW)):
                        nc.tensor.matmul(
                            pt[:, n0:n1],
                            lhsT=awb_t[:, dh * 128 : (dh + 1) * 128],
                            rhs=tTt[:, dh * SW + n0 : dh * SW + n1],
                            start=(dh == 0),
                            stop=(dh == 1),
                        )
                for h in range(2):
                    rows = slice(h * 128, (h + 1) * 128)
                    # lrelu(y) = 0.8*relu(y) + 0.2*y, y = t + s[b]
                    u8 = bk.tile([128, SW], f32, tag="u8")
                    nc.scalar.activation(
                        u8[:],
                        pt[:],
                        mybir.ActivationFunctionType.Relu,
                        bias=s08[:, h : h + 1],
                        scale=1.0 - ALPHA,
                    )
                    y2 = bk.tile([128, SW], f32, tag="y2")
                    nc.scalar.activation(
                        y2[:],
                        pt[:],
                        mybir.ActivationFunctionType.Identity,
                        bias=s02[:, h : h + 1],
                        scale=ALPHA,
                    )
                    lr = bk.tile([128, SW], f32, tag="lr")
                    nc.gpsimd.tensor_tensor(
                        out=lr[:], in0=u8[:], in1=y2[:], op=mybir.AluOpType.add
                    )
                    o = bk.tile([128, SW], f32, tag="o")
                    nc.vector.tensor_tensor(
                        out=o[:],
                        in0=lr[:],
                        in1=m8[:, h * SW : (h + 1) * SW],
                        op=mybir.AluOpType.mult,
                    )
                    nc.sync.dma_start(out.ap()[rows, cs], o[:])

    nc.compile()
    _CACHE["nc"] = nc
    return nc


def _prep_inputs(words, word_emb_table, attr_emb, a):
    words = np.ascontiguousarray(words).astype(np.int64)
    wet = np.ascontiguousarray(word_emb_table, dtype=np.float32)
    attr = np.ascontiguousarray(attr_emb, dtype=np.float32)
    a = np.ascontiguousarray(a, dtype=np.float32).reshape(-1)

    awb_rep = np.ascontiguousarray(np.repeat(a[:DW, None], 128, axis=1))
    aa_rep = np.ascontiguousarray(np.broadcast_to(a[DW:][None, :], (128, DA)))

    tblpad = np.zeros((NCORES * VS, DW), dtype=np.float32)
    tblpad[:V] = wet
    tblT_full = np.ascontiguousarray(tblpad.T)

    occ_full = np.zeros((B, NCORES * VS), dtype=np.int8)
    rows = np.repeat(np.arange(B), L)
    occ_full[rows, words.reshape(-1)] = 1

    in_maps = []
    for i in range(NCORES):
        in_maps.append(
            {
                "tblT": np.ascontiguousarray(tblT_full[:, i * VS : (i + 1) * VS]),
                "occ": np.ascontiguousarray(occ_full[:, i * VS : (i + 1) * VS]),
                "awb": awb_rep,
                "aa": aa_rep,
                "attr": attr,
            }
        )
    return in_maps


def kernel(words, word_emb_table, attr_emb, a, _trace=False, **_kw):
    nc = _build()
    in_maps = _prep_inputs(words, word_emb_table, attr_emb, a)
    res = run_bass_kernel_spmd(nc, in_maps, list(range(NCORES)), trace=_trace)
    full = np.concatenate(
        [res.results[i]["out"] for i in range(NCORES)], axis=1
    )
    out = np.ascontiguousarray(full[:, :V])
    if _trace:
        return out, res
    return out



# revision 4
# speedup vs baseline: 1.6463x; 1.6463x over previous
"""Trainium2 Bass kernel v3 for nn_AttentionLayer — fp16 table, fused Lrelu.

Math (vocab-sharded across 8 cores, VS=6400 columns each):
    out[b, v] = occ[b, v] * leaky_relu(t[v] + s[b]),
    t = table_shard @ a_w   (PE, fp16 in / f32 PSUM, t replicated over partitions)
    s = attr_emb @ a_a      (DVE mult+reduce, f32)

v3 changes vs v2 (69.7us):
  - table + a_w in fp16: halves the dominant input DMA (6.55MB -> 3.28MB)
  - single ACT pass: out_pre = Lrelu(pt + s[b]) via native leaky_relu with
    per-partition bias and alpha=0.2 (replaces 2xACT + POOL add)
  - mask multiply alternates DVE (h=0) / POOL (h=1)
  - DMA queue split: table loads on ACT's HWDGE queue, occ loads on DVE's,
    output writes on SP's — no head-of-line blocking between streams
  - all input loads hoisted with enough pool bufs to cover the full slice
"""

import numpy as np

import concourse.bass as bass
import concourse.tile as tile
from concourse import bacc, mybir
from concourse.bass_utils import run_bass_kernel_spmd

B = 256
L = 512
V = 50257
DW = 256
DA = 256
ALPHA = 0.2

NCORES = 8
VS = 6400          # vocab span per core
SW = 1280          # strip width
NS = VS // SW      # 5 strips

_CACHE = {}


def _build():
    if "nc" in _CACHE:
        return _CACHE["nc"]
    f32 = mybir.dt.float32
    f16 = mybir.dt.float16
    i8 = mybir.dt.int8

    nc = bacc.Bacc("TRN2", target_bir_lowering=False, debug=False)
    tblT = nc.declare_dram_parameter("tblT", [DW, VS], f16, isOutput=False)
    occ = nc.declare_dram_parameter("occ", [B, VS], i8, isOutput=False)
    awb = nc.declare_dram_parameter("awb", [DW, 128], f16, isOutput=False)
    aa = nc.declare_dram_parameter("aa", [128, DA], f32, isOutput=False)
    attr = nc.declare_dram_parameter("attr", [B, DA], f32, isOutput=False)
    out = nc.declare_dram_parameter("out", [B, VS], f32, isOutput=True)

    with tile.TileContext(nc) as tc:
        with (
            tc.tile_pool(name="sb", bufs=1) as sb,
            tc.tile_pool(name="tp", bufs=NS) as tp,
            tc.tile_pool(name="oc", bufs=NS) as oc,
            tc.tile_pool(name="bk", bufs=6) as bk,
            tc.tile_pool(name="pst", bufs=2, space="PSUM") as pst,
        ):
            # a_w column-replicated, one [128, 128] tile per d-half
            awb_t = sb.tile([128, 2 * 128], f16, tag="awb")
            nc.sync.dma_start(
                awb_t[:].rearrange("p (dh m) -> p dh m", m=128),
                awb.ap().rearrange("(dh p) m -> p dh m", p=128),
            )
            aa_t = sb.tile([128, DA], f32, tag="aa")
            nc.sync.dma_start(aa_t[:], aa.ap())

            # ---- hoisted strip loads on dedicated queues ----
            tTts, m8s = [], []
            for si in range(NS):
                cs = slice(si * SW, (si + 1) * SW)
                tTt = tp.tile([128, 2 * SW], f16, tag="tblT")
                nc.scalar.dma_start(
                    tTt[:].rearrange("p (dh v) -> p dh v", v=SW),
                    tblT.ap().rearrange("(dh p) v -> p dh v", p=128)[:, :, cs],
                )
                tTts.append(tTt)
                m8 = oc.tile([128, 2 * SW], i8, tag="m8")
                nc.scalar.dma_start(
                    m8[:].rearrange("p (h v) -> p h v", v=SW),
                    occ.ap().rearrange("(h p) v -> p h v", p=128)[:, :, cs],
                )
                m8s.append(m8)

            # ---- s = attr_emb @ a_a  (s_sb[:, h] holds b = h*128 + p) ----
            s_sb = sb.tile([128, 2], f32, tag="s")
            for h in range(2):
                at = sb.tile([128, DA], f32, tag=f"attr{h}")
                nc.sync.dma_start(at[:], attr.ap()[h * 128 : (h + 1) * 128, :])
                pa = sb.tile([128, DA], f32, tag=f"pa{h}")
                nc.vector.tensor_tensor(
                    out=pa[:], in0=at[:], in1=aa_t[:], op=mybir.AluOpType.mult
                )
                nc.vector.tensor_reduce(
                    out=s_sb[:, h : h + 1],
                    in_=pa[:],
                    axis=mybir.AxisListType.X,
                    op=mybir.AluOpType.add,
                )

            # ---- per strip ----
            for si in range(NS):
                cs = slice(si * SW, (si + 1) * SW)
                tTt = tTts[si]
                m8 = m8s[si]
                # t (replicated across partitions) = awb^T @ tblT, K-accumulated
                pt = pst.tile([128, SW], f32, tag="pt")
                for dh in range(2):
                    for n0, n1 in ((0, 512), (512, 1024), (1024, SW)):
                        nc.tensor.matmul(
                            pt[:, n0:n1],
                            lhsT=awb_t[:, dh * 128 : (dh + 1) * 128],
                            rhs=tTt[:, dh * SW + n0 : dh * SW + n1],
                            start=(dh == 0),
                            stop=(dh == 1),
                        )
                for h in range(2):
                    rows = slice(h * 128, (h + 1) * 128)
                    # u = leaky_relu(t + s[b]) in one ACT pass (Prelu honors
                    # alpha; Lrelu's slope is hard-baked to 0.01)
                    u = bk.tile([128, SW], f32, tag="u")
                    nc.scalar.activation(
                        u[:],
                        pt[:],
                        mybir.ActivationFunctionType.Prelu,
                        bias=s_sb[:, h : h + 1],
                        scale=1.0,
                        alpha=ALPHA,
                    )
                    # mask multiply split column-wise across DVE and POOL,
                    # each half DMA'd out independently to shorten the tail
                    o = bk.tile([128, SW], f32, tag="o")
                    HW2 = SW // 2
                    for half, eng in ((0, nc.vector), (1, nc.gpsimd)):
                        csl = slice(half * HW2, (half + 1) * HW2)
                        eng.tensor_tensor(
                            out=o[:, csl],
                            in0=u[:, csl],
                            in1=m8[:, h * SW + half * HW2 : h * SW + (half + 1) * HW2],
                            op=mybir.AluOpType.mult,
                        )
                        nc.sync.dma_start(
                            out.ap()[rows, si * SW + half * HW2 : si * SW + (half + 1) * HW2],
                            o[:, csl],
                        )

    nc.compile()
    _CACHE["nc"] = nc
    return nc


def _prep_inputs(words, word_emb_table, attr_emb, a):
    words = np.ascontiguousarray(words).astype(np.int64)
    wet = np.ascontiguousarray(word_emb_table, dtype=np.float32)
    attr = np.ascontiguousarray(attr_emb, dtype=np.float32)
    a = np.ascontiguousarray(a, dtype=np.float32).reshape(-1)

    awb_rep = np.ascontiguousarray(
        np.repeat(a[:DW, None], 128, axis=1).astype(np.float16)
    )
    aa_rep = np.ascontiguousarray(np.broadcast_to(a[DW:][None, :], (128, DA)))

    tblpad = np.zeros((NCORES * VS, DW), dtype=np.float32)
    tblpad[:V] = wet
    tblT_full = np.ascontiguousarray(tblpad.T.astype(np.float16))

    occ_full = np.zeros((B, NCORES * VS), dtype=np.int8)
    rows = np.repeat(np.arange(B), L)
    occ_full[rows, words.reshape(-1)] = 1

    in_maps = []
    for i in range(NCORES):
        in_maps.append(
            {
                "tblT": np.ascontiguousarray(tblT_full[:, i * VS : (i + 1) * VS]),
                "occ": np.ascontiguousarray(occ_full[:, i * VS : (i + 1) * VS]),
                "awb": awb_rep,
                "aa": aa_rep,
                "attr": attr,
            }
        )
    return in_maps


def kernel(words, word_emb_table, attr_emb, a, _trace=False, **_kw):
    nc = _build()
    in_maps = _prep_inputs(words, word_emb_table, attr_emb, a)
    res = run_bass_kernel_spmd(nc, in_maps, list(range(NCORES)), trace=_trace)
    full = np.concatenate(
        [res.results[i]["out"] for i in range(NCORES)], axis=1
    )
    out = np.ascontiguousarray(full[:, :V])
    if _trace:
        return out, res
    return out
